# revision 18
# baseline (speedup 1.0000x reference)
"""Distributed Trainium2 kernel for GQA sliding-window attention w/ softcap.

Sharding: 8 cores = fsdp(batch)=2 x tp(heads)=4. Core c handles batch c//4,
q-heads [4r:4r+4], kv-heads [2r:2r+2] (r = c%4). Each core computes its
partial output projection (sum over its 4 heads); host sums the 4 tp partials
per batch (the unshard step).

Design notes (v1 baseline 621us -> ~265us):
- All inputs host-packed into [128, X] DRAM blobs, loaded with few wide DMAs
  (>=8KB rows) split across the sync + scalar DMA rings in first-use order;
  the first loads are sliced so matmuls start while data streams in.
- Attention band is column-sliced: for s-block j and 512-token q-chunk c only
  the valid q columns [p0,p1)*128 are computed (25% less attn work).
- PV is computed in [tok, H] orientation (lhsT = probs slice) with a ones
  column packed into V (rhs [S, 129]) so softmax denominators fall out of the
  same matmuls; normalization is a per-partition tensor_scalar multiply and
  reciprocals run in [128,1] layout (the [1,512] layout costs 3.3us EACH).
- PSUM rule (hardware-verified): only ONE accumulation group may be open per
  PSUM bank; groups sharing a bank must run start..stop sequentially.
  Matmul PSUM regions in a shared bank are 8-byte aligned (130-float pair
  stride).
- Normalized output is transposed back to [H, tok] via PE transpose for the
  output projection; no f32 matmuls anywhere (f32 is 4x slower on PE).
- Mask multiplies run on the otherwise-idle gpsimd engine (gpsimd cannot
  touch PSUM); exp is batched into wide in-place activations over a packed
  bf16 tile; oproj psum->sbuf copies run on the vector engine.
- Schedule: attention for chunk c overlaps projection of chunk c+1 (PE-heavy
  proj hides attn's scalar-engine tanh/exp); all output projections are
  deferred into the attn3 window as PE filler emitted between PV and the
  norm->transpose chain:
  proj0 (attn0+proj1) (attn1+proj2) (attn2+proj3) (attn3+oproj0..2) oproj3.
- Output stores are pair-merged [128, 2*D] (8KB rows) alternating DMA rings;
  the final chunk stores per-tile to shorten the drain.
"""

import numpy as np
import ml_dtypes

B, T, D, H = 2, 2048, 2048, 128
NQ, NKV = 16, 8
HL, KL = 4, 2          # q heads / kv heads per core
WINDOW = 1024
SOFT_CAP = 50.0
CHUNK = 512
NC_CHUNK = T // CHUNK  # 4
NBLK = T // 128        # 16

LAST_RESULT = None

bf16 = ml_dtypes.bfloat16


def _band(c):
    """Valid s-blocks and q-column ranges for chunk c.

    Returns list of (j, p0, p1): s-block j attends to q-tiles p in [p0,p1)
    of chunk c (global q-tile tq = 4c+p; valid iff tq-8 <= j <= tq)."""
    out = []
    for j in range(max(0, 4 * c - 8), 4 * c + 4):
        delta = j - 4 * c
        p0 = max(0, delta)
        p1 = min(4, delta + 9)
        if p0 < p1:
            out.append((j, p0, p1))
    return out


def _build_graph():
    import concourse.bass as bass
    import concourse.mybir as mybir
    from concourse import bacc
    from concourse.tile import TileContext
    from contextlib import ExitStack

    dt = mybir.dt
    AF = mybir.ActivationFunctionType
    nc = bacc.Bacc()

    # wqm = wq (8192) | mtri (384); wkv2 = wk (4096) | wv (4096);
    # tab = per-chunk [cos512 | sin512] groups (f32)
    wqm = nc.declare_dram_parameter("wqm", [128, HL * 16 * 128 + 384], dt.bfloat16, isOutput=False)
    wkv2 = nc.declare_dram_parameter("wkv2", [128, KL * 16 * 128 + 16 * 256], dt.bfloat16, isOutput=False)
    wo = nc.declare_dram_parameter("wo", [128, HL * D], dt.bfloat16, isOutput=False)
    tab = nc.declare_dram_parameter("tab", [128, 2 * T], dt.float32, isOutput=False)
    xt = nc.declare_dram_parameter("xt", [128, NC_CHUNK * 16 * CHUNK], dt.bfloat16, isOutput=False)
    out = nc.declare_dram_parameter("out", [128, NBLK * D], dt.bfloat16, isOutput=True)

    with TileContext(nc) as tc, ExitStack() as ctx:
        p_w = ctx.enter_context(tc.tile_pool(name="w", bufs=1))
        p_xt = ctx.enter_context(tc.tile_pool(name="xt", bufs=2))
        p_qt = ctx.enter_context(tc.tile_pool(name="qt", bufs=16))
        p_kt = ctx.enter_context(tc.tile_pool(name="kt", bufs=8))
        p_v = ctx.enter_context(tc.tile_pool(name="v", bufs=16))
        p_rt = ctx.enter_context(tc.tile_pool(name="rt", bufs=4))
        p_e = ctx.enter_context(tc.tile_pool(name="e", bufs=2))
        p_rc = ctx.enter_context(tc.tile_pool(name="rc", bufs=8))
        p_eb = ctx.enter_context(tc.tile_pool(name="eb", bufs=2))
        p_enc = ctx.enter_context(tc.tile_pool(name="enc", bufs=16))
        p_ost = ctx.enter_context(tc.tile_pool(name="ost", bufs=2))
        ps_mm = ctx.enter_context(tc.tile_pool(name="psmm", bufs=2, space="PSUM"))
        ps_lp = ctx.enter_context(tc.tile_pool(name="pslp", bufs=3, space="PSUM"))
        ps_pv = ctx.enter_context(tc.tile_pool(name="pspv", bufs=2, space="PSUM"))
        ps_tr = ctx.enter_context(tc.tile_pool(name="pstr", bufs=1, space="PSUM"))

        # --- persistent weight / table tiles (few wide DMAs; rings split) ---
        wqm_sb = p_w.tile([128, HL * 16 * 128 + 384], dt.bfloat16, tag="wqm", name="wqm_sb")
        wkv_sb = p_w.tile([128, KL * 16 * 128 + 16 * 256], dt.bfloat16, tag="wkv", name="wkv_sb")
        wq_sb = wqm_sb[:, 0:HL * 16 * 128]
        maskd = wqm_sb[:, 8192:8320]   # keep s<=t (diagonal block)
        maskl = wqm_sb[:, 8320:8448]   # keep s>t (window corner block)
        idn = wqm_sb[:, 8448:8576]     # identity (PE transpose)
        wk_sb = wkv_sb[:, 0:KL * 16 * 128]
        wv_sb = wkv_sb[:, 4096:8192]
        wo_sb = p_w.tile([128, HL * D], dt.bfloat16, tag="wo", name="wo_sb")
        tab_sb = p_w.tile([128, 2 * T], dt.float32, tag="tab", name="tab_sb")

        # sync ring: weights + tables; scalar ring: xt chunks (parallel rings,
        # descriptor-efficient >=8KB rows per DMA)
        xt_sb = {}
        for c in range(NC_CHUNK):
            xt_sb[c] = p_xt.tile([128, 16 * CHUNK], dt.bfloat16, tag="xt", name="xtt")
        nc.sync.dma_start(xt_sb[0][:, 0:2048], xt[:, 0:2048])
        nc.scalar.dma_start(wqm_sb[:, 0:2048], wqm[:, 0:2048])
        nc.sync.dma_start(xt_sb[0][:, 2048:4096], xt[:, 2048:4096])
        nc.scalar.dma_start(xt_sb[0][:, 4096:6144], xt[:, 4096:6144])
        nc.sync.dma_start(wqm_sb[:, 2048:4096], wqm[:, 2048:4096])
        nc.scalar.dma_start(xt_sb[0][:, 6144:8192], xt[:, 6144:8192])
        nc.sync.dma_start(wqm_sb[:, 4096:8576], wqm[:, 4096:8576])
        nc.scalar.dma_start(tab_sb[:, 0:1024], tab[:, 0:1024])
        nc.sync.dma_start(wkv_sb[:], wkv2[:])
        nc.scalar.dma_start(xt_sb[1][:], xt[:, 8192:16384])
        nc.sync.dma_start(tab_sb[:, 1024:4096], tab[:, 1024:4096])
        nc.scalar.dma_start(xt_sb[2][:], xt[:, 16384:24576])
        nc.sync.dma_start(wo_sb[:], wo[:])
        nc.scalar.dma_start(xt_sb[3][:], xt[:, 24576:32768])

        qt_sb = {}
        kt_sb = {}
        v_sb = [p_v.tile([128, 258], dt.bfloat16, tag="v", name="vt") for _ in range(NBLK)]
        # ones columns for the packed denominators; written once, the v
        # copies below never touch cols 128/257.
        for i in range(NBLK):
            nc.vector.memset(v_sb[i][:, 128:129], 1.0)
            nc.vector.memset(v_sb[i][:, 257:258], 1.0)
        enc_sb = {}

        def rope(ps, dst, c):
            cosv = tab_sb[:, c * 1024:c * 1024 + 512]
            sinv = tab_sb[:, c * 1024 + 512:(c + 1) * 1024]
            m1 = p_rt.tile([128, CHUNK], dt.float32, tag="rt", name="m1")
            m2 = p_rt.tile([128, CHUNK], dt.float32, tag="rt", name="m2")
            nc.vector.tensor_mul(m1[:], ps[:], cosv)
            nc.vector.tensor_mul(m2[0:64, :], ps[64:128, :], sinv[0:64, :])
            nc.vector.tensor_mul(m2[64:128, :], ps[0:64, :], sinv[64:128, :])
            nc.vector.tensor_add(dst[:], m1[:], m2[:])

        def proj_chunk_part(c, kind, idxs):
            xts = xt_sb[c]
            if kind == "q":
                for h in idxs:
                    ps = ps_mm.tile([128, CHUNK], dt.float32, tag="ps", name="psq")
                    for d in range(16):
                        nc.tensor.matmul(ps[:], wq_sb[:, (h * 16 + d) * 128:(h * 16 + d + 1) * 128],
                                         xts[:, d * CHUNK:(d + 1) * CHUNK],
                                         start=(d == 0), stop=(d == 15))
                    qt = p_qt.tile([128, CHUNK], dt.bfloat16, tag="qt", name="qtt")
                    rope(ps, qt, c)
                    qt_sb[(h, c)] = qt
            elif kind == "k":
                for h in idxs:
                    ps = ps_mm.tile([128, CHUNK], dt.float32, tag="ps", name="psk")
                    for d in range(16):
                        nc.tensor.matmul(ps[:], wk_sb[:, (h * 16 + d) * 128:(h * 16 + d + 1) * 128],
                                         xts[:, d * CHUNK:(d + 1) * CHUNK],
                                         start=(d == 0), stop=(d == 15))
                    kt = p_kt.tile([128, CHUNK], dt.bfloat16, tag="kt", name="ktt")
                    rope(ps, kt, c)
                    kt_sb[(h, c)] = kt
            else:
                for p in idxs:
                    ps = ps_mm.tile([128, 256], dt.float32, tag="ps", name="psv")
                    for d in range(16):
                        nc.tensor.matmul(ps[:], xts[:, d * CHUNK + p * 128:d * CHUNK + (p + 1) * 128],
                                         wv_sb[:, d * 256:(d + 1) * 256],
                                         start=(d == 0), stop=(d == 15))
                    vt = v_sb[c * 4 + p]
                    nc.vector.tensor_copy(vt[:, 0:128], ps[:, 0:128])
                    nc.vector.tensor_copy(vt[:, 129:257], ps[:, 128:256])

        def proj_chunk(c):
            proj_chunk_part(c, "q", [0, 1, 2, 3])
            proj_chunk_part(c, "k", [0, 1])
            proj_chunk_part(c, "v", [0, 1, 2, 3])

        ost_cur = [None]

        def oproj_piece(c, p):
            """Output projection for q-tile p of chunk c: [128 tok, D].
            Stores are merged per p-pair ([128, 2D] = 8KB rows) and alternate
            between the two DMA rings."""
            tq = c * 4 + p
            if p % 2 == 0:
                ost_cur[0] = p_ost.tile([128, 2 * D], dt.bfloat16, tag="ost", name="ot")
            ot = ost_cur[0]
            o0 = (p % 2) * D
            for dc in range(4):
                ps = ps_mm.tile([128, 512], dt.float32, tag="ps", name="pso")
                for h in range(HL):
                    nc.tensor.matmul(ps[:], enc_sb[(c, h)][:, p * 128:(p + 1) * 128],
                                     wo_sb[:, h * D + dc * 512:h * D + (dc + 1) * 512],
                                     start=(h == 0), stop=(h == HL - 1))
                nc.vector.tensor_copy(ot[:, o0 + dc * 512:o0 + (dc + 1) * 512], ps[:])
            if c == 3:
                eng = nc.sync if tq % 2 == 0 else nc.scalar
                eng.dma_start(out[:, tq * D:(tq + 1) * D], ot[:, o0:o0 + D])
            elif p % 2 == 1:
                eng = nc.sync if (tq // 2) % 2 == 0 else nc.scalar
                eng.dma_start(out[:, (tq - 1) * D:(tq + 1) * D], ot[:])

        def attn_chunk(c, extra=None):
            """extra: optional list of thunks; extra[h] is emitted after head h
            to give the PE queue filler work while ACT catches up."""
            band = _band(c)
            offs = {}
            o = 0
            for (j, p0, p1) in band:
                offs[j] = o
                o += (p1 - p0) * 128
            wtot = o
            # groups of up to 2 consecutive j for wide exp: finer exp ops
            # interleave with tanh in the scalar queue, so QKs are not
            # throttled waiting for lp slots behind a long exp
            groups = [band[i:i + 2] for i in range(0, len(band), 2)]

            for h in range(HL):
                kv = h // 2
                qt = qt_sb[(h, c)]
                ew = p_e.tile([128, wtot], dt.bfloat16, tag="e", name="ew")
                pvb = {}
                pvb[0] = ps_pv.tile([128, 260], dt.float32, tag="pv", name="pv01")
                pvb[1] = ps_pv.tile([128, 260], dt.float32, tag="pv", name="pv23")

                def emit_qk_group(g):
                    for (j, p0, p1) in g:
                        w = (p1 - p0) * 128
                        lp = ps_lp.tile([128, w], dt.float32, tag="lp", name="lp")
                        nc.tensor.matmul(lp[:], kt_sb[(kv, j // 4)][:, (j % 4) * 128:(j % 4 + 1) * 128],
                                         qt[:, p0 * 128:p1 * 128], start=True, stop=True)
                        # tanh (with softcap prescale) -> bf16 into the wide tile
                        nc.scalar.activation(ew[:, offs[j]:offs[j] + w], lp[:],
                                             AF.Tanh, scale=1.0 / SOFT_CAP)

                def emit_exp_group(g):
                    j0, p00, _ = g[0]
                    jl, pl0, pl1 = g[-1]
                    lo = offs[j0]
                    hi = offs[jl] + (pl1 - pl0) * 128
                    nc.scalar.activation(ew[:, lo:hi], ew[:, lo:hi], AF.Exp, scale=SOFT_CAP)
                    # masks for this group's partial blocks (idle gpsimd)
                    for (j, p0, p1) in g:
                        delta = j - 4 * c
                        if 0 <= delta <= 3:   # diagonal: first subcolumn
                            r = offs[j]
                            nc.gpsimd.tensor_mul(ew[:, r:r + 128], ew[:, r:r + 128], maskd)
                        if -8 <= delta <= -5:  # window corner: last subcolumn
                            r = offs[j] + (p1 - p0) * 128 - 128
                            nc.gpsimd.tensor_mul(ew[:, r:r + 128], ew[:, r:r + 128], maskl)

                def emit_pv_p(p):
                    # one complete accumulation group per q-tile p: PSUM allows
                    # only one OPEN group per bank, so groups sharing a bank
                    # must run start..stop sequentially, never interleaved.
                    tq = 4 * c + p
                    col = (p % 2) * 130
                    jfirst = max(0, tq - 8)
                    for j in range(jfirst, tq + 1):
                        p0 = max(0, j - 4 * c)
                        nc.tensor.matmul(
                            pvb[p // 2][:, col:col + 129],
                            ew[:, offs[j] + (p - p0) * 128:offs[j] + (p - p0 + 1) * 128],
                            v_sb[j][:, kv * 129:kv * 129 + 129],
                            start=(j == jfirst), stop=(j == tq),
                            skip_group_check=True)

                for g in groups:
                    emit_qk_group(g)
                    emit_exp_group(g)
                for p in range(4):
                    emit_pv_p(p)
                # PE filler between pv and the norm->transpose chain so the
                # tensor queue is not blocked on the DVE latency
                if extra is not None and h < len(extra):
                    extra[h]()
                    extra[h] = None

                # normalize (per-partition scalar = 1/denominator), then
                # transpose back to [H, tok] for the output projection.
                eb = p_eb.tile([128, CHUNK], dt.bfloat16, tag="eb", name="eb")
                et = ps_tr.tile([128, CHUNK], dt.bfloat16, tag="et", name="et")
                for p in range(4):
                    col = (p % 2) * 130
                    rc = p_rc.tile([128, 1], dt.float32, tag="rc", name="rc")
                    nc.vector.reciprocal(rc[:], pvb[p // 2][:, col + 128:col + 129])
                    nc.vector.tensor_scalar_mul(eb[:, p * 128:(p + 1) * 128],
                                                pvb[p // 2][:, col:col + 128], rc[:])
                    nc.tensor.transpose(et[:, p * 128:(p + 1) * 128],
                                        eb[:, p * 128:(p + 1) * 128], idn)
                enc = p_enc.tile([128, CHUNK], dt.bfloat16, tag="enc", name="enct")
                nc.vector.tensor_copy(enc[:], et[:])
                enc_sb[(c, h)] = enc

        def proj_pieces(c):
            """proj_chunk split into 4 thunks (PE filler between attn heads)."""
            return [
                lambda: proj_chunk_part(c, "q", [0, 1]),
                lambda: proj_chunk_part(c, "q", [2, 3]),
                lambda: proj_chunk_part(c, "k", [0, 1]),
                lambda: proj_chunk_part(c, "v", [0, 1, 2, 3]),
            ]

        proj_chunk(0)
        attn_chunk(0, extra=proj_pieces(1))
        attn_chunk(1, extra=proj_pieces(2))
        attn_chunk(2, extra=proj_pieces(3))
        attn_chunk(3, extra=[
            lambda: (oproj_piece(0, 0), oproj_piece(0, 1), oproj_piece(0, 2), oproj_piece(0, 3)),
            lambda: (oproj_piece(1, 0), oproj_piece(1, 1), oproj_piece(1, 2), oproj_piece(1, 3)),
            lambda: (oproj_piece(2, 0), oproj_piece(2, 1)),
            lambda: (oproj_piece(2, 2), oproj_piece(2, 3)),
        ])
        for p in range(4):
            oproj_piece(3, p)

    nc.compile()
    return nc


def _rope_tables(positions):
    """[128, 2T] chunk-grouped: per chunk c, cols [c*1024, c*1024+512) = cos
    stack [cos;cos], cols [c*1024+512, (c+1)*1024) = sin stack [-sin;sin]."""
    frac = 2.0 * np.arange(64) / H
    timescale = 10000.0 ** frac
    ang = positions[None, :].astype(np.float64) / timescale[:, None]
    cos = np.cos(ang).astype(np.float32)
    sin = np.sin(ang).astype(np.float32)
    tabc = np.concatenate([cos, cos], axis=0)           # [128, T]
    tabs = np.concatenate([-sin, sin], axis=0)          # [128, T]
    tab = np.zeros((128, 2 * T), dtype=np.float32)
    for c in range(NC_CHUNK):
        tab[:, c * 1024:c * 1024 + 512] = tabc[:, c * 512:(c + 1) * 512]
        tab[:, c * 1024 + 512:(c + 1) * 1024] = tabs[:, c * 512:(c + 1) * 512]
    return np.ascontiguousarray(tab)


def kernel(x, segment_pos, attn_mask, wq, wkv, wo):
    global LAST_RESULT
    x = np.asarray(x)
    segment_pos = np.asarray(segment_pos)
    wq = np.asarray(wq)
    wkv = np.asarray(wkv)
    wo = np.asarray(wo)

    from concourse.bass_utils import run_bass_kernel_spmd

    nc = _build_graph()
    scale = H ** -0.5

    sig = np.arange(128)[:, None]
    tau = np.arange(128)[None, :]
    mtri = np.concatenate([
        (sig <= tau).astype(np.float32),
        (sig > tau).astype(np.float32),
        np.eye(128, dtype=np.float32),
    ], axis=1).astype(bf16)

    in_maps = []
    for core in range(8):
        b, r = core // 4, core % 4
        tab = _rope_tables(segment_pos[b])
        # wq: [4,2048,128] -> [128p, (h,d)*128] with d-tile on partitions
        wq_p = (wq[4 * r:4 * r + 4] * scale).reshape(4, 16, 128, 128)
        wq_p = np.ascontiguousarray(wq_p.transpose(2, 0, 1, 3)).reshape(128, -1)
        wk_p = wkv[0, 2 * r:2 * r + 2].reshape(2, 16, 128, 128)
        wk_p = np.ascontiguousarray(wk_p.transpose(2, 0, 1, 3)).reshape(128, -1)
        # wv: [2,2048,128] -> [128p, d*(kv,j)] (x-side proj: [d-tile, 256])
        wv_p = wkv[1, 2 * r:2 * r + 2]                  # (kv, dfull, j)
        wv_p = wv_p.reshape(2, 16, 128, 128)            # (kv, d, p, j)
        wv_p = np.ascontiguousarray(wv_p.transpose(2, 1, 0, 3)).reshape(128, -1)
        wo_p = wo[4 * r:4 * r + 4]                      # (h, H=128, D)
        wo_p = np.ascontiguousarray(wo_p.transpose(1, 0, 2)).reshape(128, -1)
        xt_p = x[b].reshape(4, 512, 16, 128)            # (c, t, d, p)
        xt_p = np.ascontiguousarray(xt_p.transpose(3, 0, 2, 1)).reshape(128, -1)
        in_maps.append({
            "wqm": np.concatenate([wq_p.astype(bf16), mtri], axis=1),
            "wkv2": np.concatenate([wk_p.astype(bf16), wv_p.astype(bf16)], axis=1),
            "wo": wo_p.astype(bf16),
            "tab": tab,
            "xt": xt_p.astype(bf16),
        })

    res = run_bass_kernel_spmd(nc, in_maps, core_ids=list(range(8)))
    LAST_RESULT = res
    out = np.zeros((B, T, D), dtype=np.float32)
    for core in range(8):
        o = np.asarray(res.results[core]["out"], dtype=np.float32)
        o = o.reshape(128, 16, D).transpose(1, 0, 2).reshape(T, D)
        out[core // 4] += o
    return out
